# revision 1
# baseline (speedup 1.0000x reference)
"""Multi-depth attention (BaseMAWAttention) Trainium2 kernel — v2.

Sharding: 8 cores = 4 batches x 2 head-halves (6 heads each). Each core
computes its (batch, head-half) slice end-to-end: QKV projections,
per-(head,depth) scores, softmax, AV. No collectives.

Device layouts (per core):
  hsT   [128, 6, 512]        hidden_states[b].T, k-major (ki, ko, s), bf16
  wq/wk [24, 128, 6, 128]    permuted-transposed Wqd/Wkd slices: (fc, ki, ko, f)
                             f' = h*512 + d*64 + e  (head, depth, e ordered)
  wv    [128, 6, 384]        Wv slice transposed (ki, ko, (h e))
  qb/kb [128, 24]            permuted bias slices (per-partition, per f-chunk)
  out   [6, 4, 2, 128, 4, 64] (h, qchunk, dgrp, qp, dsub, e), fp32

The program is an explicit software pipeline over 12 units (head,
depth-group). Per "tick" (one score-PSUM tile = 2 depths x 512 q):
the 2 score matmuls back-to-back (they run concurrently on PE row
tiles 0-63/64-127 — measured 162ns each vs 428ns solo; a K=64 matmul
solo streams at HALF rate), the exp of that tile on the Act engine,
then 8 AV matmuls of the PREVIOUS unit and 3 projection matmuls of
the NEXT head interleaved, so the PE never waits on exp and short AV
streams sit between long ones.

Measured HW costs/core (microbenched): proj mm (K=128,N=512) ~274ns
x288, score pair ~2x280ns x96, AV mm (K=128,N=65) ~50ns x768, exp
[128,1024] ~1.28us x96 (Act engine, the second-largest engine load).
PSUM budget (8 banks): sc 2x2 + pp 2x1 + av 2x1.

DMA: weights stream on the SP HWDGE queue (4 chunks prefetch ahead);
hsT/wv consts on the Act HWDGE queue; the output leaves per unit as
ONE [128,4,4,64] bf16 DMA on the GPSIMD SWDGE queue (DRAM layout
matches SBUF, 2KB descriptors) — three queues so no single DMA queue
serializes (out in fp32 on one queue measured 151us alone).

All matmuls bf16 (fp32 runs at 1/4 rate on the PE; fp8 fails the 2e-2
accuracy gate, and fp8-DoubleRow does not lower under this compile
path). Softmax needs no max-subtraction: scores are O(1) and the mask
enters as an additive bias, which cancels in the softmax ratio exactly
like the reference's where(mask==0,-1e9)+max-subtract. The AV matmul
uses V augmented with a ones column so the same matmul yields the
softmax denominator in column 64; DVE reciprocal + broadcast-multiply
normalizes into bf16 (host upcasts to fp32).
"""

import os
import sys

import numpy as np

try:
    import concourse.bass as bass  # noqa: F401
except ImportError:
    sys.path.insert(0, "/opt/trn_rl_repo")

import ml_dtypes

HIDDEN = 768
HEADS = 12
HD = 64
DEPTH = 8
B = 4
S = 512
N_CORES = 8
HB = HEADS // 2          # heads per core
P = 128
NKC = HIDDEN // P        # 6 contraction chunks
FC = HB * DEPTH * HD // P  # 24 feature chunks of Q'/K'
KSC = S // P             # 4 key/seq chunks
F = FC * P               # 3072

_BF = ml_dtypes.bfloat16

_cache = {}


def _build(use_mask, use_qk_bias, reps=1, probe=None, tune=0):
    import contextlib

    import concourse.bacc as bacc
    import concourse.mybir as mybir
    import concourse.tile as tile

    f32 = mybir.dt.float32
    bf = mybir.dt.bfloat16
    Exp = mybir.ActivationFunctionType.Exp

    nc = bacc.Bacc(
        "TRN2", target_bir_lowering=False, debug=False, num_devices=N_CORES
    )
    hsT_d = nc.dram_tensor("hsT", [P, NKC, S], bf, kind="ExternalInput")
    wq_d = nc.dram_tensor("wq", [FC, P, NKC, P], bf, kind="ExternalInput")
    wk_d = nc.dram_tensor("wk", [FC, P, NKC, P], bf, kind="ExternalInput")
    wv_d = nc.dram_tensor("wv", [P, NKC, HB * HD], bf, kind="ExternalInput")
    if use_qk_bias:
        qb_d = nc.dram_tensor("qb", [P, FC], f32, kind="ExternalInput")
        kb_d = nc.dram_tensor("kb", [P, FC], f32, kind="ExternalInput")
    if use_mask:
        mb_d = nc.dram_tensor("mb", [P, KSC], f32, kind="ExternalInput")
    out_d = nc.dram_tensor(
        "out", [HB, 2, P, KSC, 4, HD], bf, kind="ExternalOutput"
    )

    with tile.TileContext(nc) as tc:
        with (
            tc.tile_pool(name="consts", bufs=1) as consts,
            tc.tile_pool(name="wts", bufs=(6 if tune == 9 else 8)) as wts,
            tc.tile_pool(name="qk", bufs=(3 if tune == 8 else 2)) as qk,
            tc.tile_pool(name="ep", bufs={8: 6, 9: 4}.get(tune, 5)) as ep,
            tc.tile_pool(name="ob", bufs={8: 6, 9: 3}.get(tune, 4)) as ob,
            tc.tile_pool(
                name="pp", bufs=(1 if tune in (1, 4) else 2), space="PSUM"
            ) as ps_pp,
            tc.tile_pool(
                name="sc", bufs=(3 if tune in (1, 4) else 2), space="PSUM"
            ) as ps_sc,
            tc.tile_pool(
                name="av", bufs=(1 if tune in (1, 4) else 2), space="PSUM"
            ) as ps_av,
            tc.For_i(0, reps, 1) if reps > 1 else contextlib.nullcontext(),
        ):
            hsT_sb = consts.tile([P, NKC, S], bf, tag="hsT")
            nc.scalar.dma_start(hsT_sb[:], hsT_d.ap())
            wv_sb = consts.tile([P, NKC, HB * HD], bf, tag="wv")
            nc.scalar.dma_start(wv_sb[:], wv_d.ap())
            if use_qk_bias:
                qb_sb = consts.tile([P, FC], f32, tag="qb")
                nc.sync.dma_start(qb_sb[:], qb_d.ap())
                kb_sb = consts.tile([P, FC], f32, tag="kb")
                nc.sync.dma_start(kb_sb[:], kb_d.ap())
            if use_mask:
                mb_sb = consts.tile([P, KSC], f32, tag="mb")
                nc.sync.dma_start(mb_sb[:], mb_d.ap())

            v_sb = consts.tile([P, KSC, HB, HD + 1], bf, tag="v")
            nc.vector.memset(v_sb[:, :, :, HD : HD + 1], 1.0)

            # ---- feeders: incremental issue of one PE matmul per step ----

            class ProjFeeder:
                """Q'/K' projections for one head: 8 chunks x 6 matmuls."""

                def __init__(self, h):
                    self.h = h
                    self.q_t = qk.tile([P, 4, S], bf, tag="q")
                    self.k_t = qk.tile([P, 4, S], bf, tag="k")
                    self.chunks = [
                        (w_d, dst, b_name, lc)
                        for lc in range(4)
                        for (w_d, dst, b_name) in (
                            (wq_d, self.q_t, "q"),
                            (wk_d, self.k_t, "k"),
                        )
                    ]
                    self.wt = [None] * 8
                    self.ci = 0
                    self.mmi = 0
                    self.pref = 0
                    self.ps = None
                    while self.pref < 4:
                        self._prefetch()

                def _prefetch(self):
                    w_d, dst, b_name, lc = self.chunks[self.pref]
                    c = self.h * 4 + lc
                    wt = wts.tile([P, NKC, P], bf, tag="w")
                    if probe in ("dmaw_none", "projonly_nodma", "projonly_baremm"):
                        nc.vector.memset(wt[:, :, 0:1], 0.01)
                    else:
                        nc.sync.dma_start(wt[:], w_d.ap()[c])
                    self.wt[self.pref] = wt
                    self.pref += 1

                def step(self):
                    if self.ci >= 8:
                        return False
                    w_d, dst, b_name, lc = self.chunks[self.ci]
                    if self.mmi == 0:
                        while self.pref < min(8, self.ci + 7):
                            self._prefetch()
                        self.ps = ps_pp.tile([P, S], f32, tag="pp", name="pp")
                    ko = self.mmi
                    nc.tensor.matmul(
                        self.ps[:],
                        self.wt[self.ci][:, ko, :],
                        hsT_sb[:, ko, :],
                        start=(ko == 0),
                        stop=(ko == NKC - 1),
                    )
                    self.mmi += 1
                    if self.mmi == NKC:
                        if probe in ("projonly_nocopy", "projonly_baremm"):
                            pass
                        elif use_qk_bias:
                            b_sb = qb_sb if b_name == "q" else kb_sb
                            c = self.h * 4 + lc
                            nc.vector.tensor_scalar_add(
                                dst[:, lc, :], self.ps[:], b_sb[:, c : c + 1]
                            )
                        elif tune == 2:
                            nc.scalar.copy(dst[:, lc, :], self.ps[:])
                        else:
                            nc.vector.tensor_copy(dst[:, lc, :], self.ps[:])
                        self.ci += 1
                        self.mmi = 0
                    return True

            class VProjFeeder:
                """V projection: 4 seq-chunks x 6 matmuls into v_sb."""

                def __init__(self):
                    self.sc_ = 0
                    self.mmi = 0
                    self.ps = None

                def step(self):
                    if self.sc_ >= KSC:
                        return False
                    if self.mmi == 0:
                        self.ps = ps_pp.tile([P, S], f32, tag="pp", name="pp")
                    ko = self.mmi
                    nc.tensor.matmul(
                        self.ps[:, : HB * HD],
                        hsT_sb[:, ko, self.sc_ * P : (self.sc_ + 1) * P],
                        wv_sb[:, ko, :],
                        start=(ko == 0),
                        stop=(ko == NKC - 1),
                    )
                    self.mmi += 1
                    if self.mmi == NKC:
                        nc.vector.tensor_copy(
                            v_sb[:, self.sc_, :, 0:HD],
                            self.ps[:, : HB * HD].rearrange(
                                "p (h e) -> p h e", e=HD
                            ),
                        )
                        self.sc_ += 1
                        self.mmi = 0
                    return True

            class AvFeeder:
                """AV for one unit (h, dgrp): 4 qc x 4 i x 4 ksc matmuls,
                then reciprocal-normalize + DMA out per qc."""

                def __init__(self, h, dgrp, e_pairs):
                    self.h = h
                    self.dgrp = dgrp
                    self.e_pairs = e_pairs
                    self.items = [
                        (qc, i, ksc)
                        for qc in range(KSC)
                        for i in range(4)
                        for ksc in range(KSC)
                    ]
                    self.n = 0
                    self.av = None
                    self.o_unit = ob.tile(
                        [P, KSC, 4, HD], bf, tag="o", name="o_unit"
                    )

                def step(self):
                    if self.n >= len(self.items):
                        return False
                    qc, i, ksc = self.items[self.n]
                    if i == 0 and ksc == 0:
                        self.av = ps_av.tile([P, 4, HD + 1], f32, tag="av", name="av")
                    pi, pd = i // 2, i % 2
                    nc.tensor.matmul(
                        self.av[:, i, :],
                        self.e_pairs[pi][:, ksc, pd, qc * P : (qc + 1) * P],
                        v_sb[:, ksc, self.h, :],
                        start=(ksc == 0),
                        stop=(ksc == KSC - 1),
                    )
                    self.n += 1
                    if i == 3 and ksc == KSC - 1:
                        r = ob.tile([P, 4], f32, tag="r")
                        nc.vector.reciprocal(r[:], self.av[:, :, HD])
                        nc.vector.tensor_mul(
                            self.o_unit[:, qc, :, :],
                            self.av[:, :, 0:HD],
                            r[:, :, None].to_broadcast([P, 4, HD]),
                        )
                        if qc == KSC - 1 and probe != "dmao_none":
                            nc.gpsimd.dma_start(
                                out_d.ap()[self.h, self.dgrp], self.o_unit[:]
                            )
                    return True

            class FillerChain:
                def __init__(self, feeders):
                    self.feeders = list(feeders)

                def append(self, f):
                    self.feeders.append(f)

                def step(self):
                    while self.feeders:
                        if self.feeders[0].step():
                            return True
                        self.feeders.pop(0)
                    return False

            # ---- schedule ----
            proj = {h: None for h in range(HB)}
            proj[0] = ProjFeeder(0)
            filler = FillerChain([proj[0], VProjFeeder()])
            # prologue: chunks (q,lc0),(k,lc0) of head 0 = 12 mm + copies
            for _ in range(12):
                filler.step()

            av_f = None  # AV feeder of previous unit
            units = [(h, dgrp) for h in range(HB) for dgrp in range(2)]
            for ui, (h, dgrp) in enumerate(units):
                if dgrp == 0 and h + 1 < HB:
                    proj[h + 1] = ProjFeeder(h + 1)
                    filler.append(proj[h + 1])
                q_t, k_t = proj[h].q_t, proj[h].k_t
                e_pairs = [None, None]

                def slotA():
                    if av_f is not None and av_f.step():
                        return
                    filler.step()

                def slotF():
                    if filler.step():
                        return
                    if av_f is not None:
                        av_f.step()

                def issue_exp(sps, pi, ksc):
                    if probe == "noexp_noav":
                        if ksc == 0:
                            nc.vector.memset(e_pairs[pi][:, :, :, 0:4], 0.5)
                    elif probe == "exphalf":
                        nc.scalar.activation(
                            e_pairs[pi][:, ksc, 0, :],
                            sps[:, 0, :],
                            Exp,
                            scale=0.125,
                        )
                    elif use_mask:
                        nc.scalar.activation(
                            e_pairs[pi][:, ksc, :, :],
                            sps[:],
                            Exp,
                            bias=mb_sb[:, ksc : ksc + 1],
                            scale=0.125,
                        )
                    else:
                        nc.scalar.activation(
                            e_pairs[pi][:, ksc].rearrange("p a b -> p (a b)"),
                            sps[:].rearrange("p a b -> p (a b)"),
                            Exp,
                            scale=0.125,
                        )

                def issue_sc(tick):
                    pi, ksc = divmod(tick, 4)
                    lc = dgrp * 2 + pi
                    if ksc == 0:
                        e_pairs[pi] = ep.tile(
                            [P, KSC, 2, S], bf, tag="exp", name="e_p"
                        )
                        if probe == "exphalf":
                            nc.vector.memset(e_pairs[pi][:, :, 1, :], 0.002)
                    sps = ps_sc.tile([P, 2, S], f32, tag="sc")
                    for pd in range(2):
                        base = pd * 64
                        nc.tensor.matmul(
                            sps[:, pd, :],
                            k_t[base : base + 64, lc, ksc * P : (ksc + 1) * P],
                            q_t[base : base + 64, lc, :],
                            start=True,
                            stop=True,
                        )
                    return sps, pi, ksc

                if tune == 4:
                    for burst in range(4):
                        if probe is not None and probe.startswith("projonly"):
                            for _ in range(26):
                                slotA()
                            continue
                        a = issue_sc(burst * 2)
                        b = issue_sc(burst * 2 + 1)
                        issue_exp(*a)
                        issue_exp(*b)
                        for s in range(22):
                            if s in (4, 7, 10, 15, 18, 21):
                                slotF()
                            else:
                                slotA()
                    if probe in ("noav", "noexp_noav") or (
                        probe is not None and probe.startswith("projonly")
                    ):
                        pass
                    else:
                        av_f = AvFeeder(h, dgrp, e_pairs)
                    continue

                for tick in range(8):
                    pi, ksc = divmod(tick, 4)
                    lc = dgrp * 2 + pi
                    if probe is not None and probe.startswith("projonly"):
                        for _ in range(13):
                            slotA()
                        continue
                    if ksc == 0:
                        e_pairs[pi] = ep.tile(
                            [P, KSC, 2, S], bf, tag="exp", name="e_p"
                        )
                        if probe == "exphalf":
                            nc.vector.memset(e_pairs[pi][:, :, 1, :], 0.002)
                    sps = ps_sc.tile([P, 2, S], f32, tag="sc")
                    for pd in range(2):
                        base = pd * 64
                        nc.tensor.matmul(
                            sps[:, pd, :],
                            k_t[base : base + 64, lc, ksc * P : (ksc + 1) * P],
                            q_t[base : base + 64, lc, :],
                            start=True,
                            stop=True,
                        )
                    # exp on Act; mask bias cancels in softmax
                    if probe == "noexp_noav":
                        if ksc == 0:
                            nc.vector.memset(e_pairs[pi][:, :, :, 0:4], 0.5)
                    elif probe == "exphalf":
                        nc.scalar.activation(
                            e_pairs[pi][:, ksc, 0, :],
                            sps[:, 0, :],
                            Exp,
                            scale=0.125,
                        )
                    elif use_mask:
                        nc.scalar.activation(
                            e_pairs[pi][:, ksc, :, :],
                            sps[:],
                            Exp,
                            bias=mb_sb[:, ksc : ksc + 1],
                            scale=0.125,
                        )
                    else:
                        nc.scalar.activation(
                            e_pairs[pi][:, ksc].rearrange("p a b -> p (a b)"),
                            sps[:].rearrange("p a b -> p (a b)"),
                            Exp,
                            scale=0.125,
                        )
                    pat = "FAAFAAAAFAA" if tune == 10 else "AAAAFAAFAAF"
                    for ch in pat:
                        if ch == "F":
                            slotF()
                        else:
                            slotA()
                if probe in ("noav", "noexp_noav") or (
                    probe is not None and probe.startswith("projonly")
                ):
                    pass
                else:
                    av_f = AvFeeder(h, dgrp, e_pairs)
            # epilogue: drain last unit's AV and any leftover fillers
            if av_f is not None:
                while av_f.step():
                    pass
            while filler.step():
                pass

    nc.compile()
    return nc


def _get_program(use_mask, use_qk_bias):
    key = (use_mask, use_qk_bias)
    if key not in _cache:
        _cache[key] = _build(use_mask, use_qk_bias)
    return _cache[key]


def _perm_idx(h0):
    # f' = h*512 + d*64 + e maps to original row ((h0+h)*64+e)*8 + d
    idx = np.empty(F, dtype=np.int64)
    f = 0
    for h in range(HB):
        for d in range(DEPTH):
            for e in range(HD):
                idx[f] = ((h0 + h) * HD + e) * DEPTH + d
                f += 1
    return idx


def _prep_w(Wd, idx):
    # [6144,768] -> permuted rows [3072,768] -> (fc, ki, ko, f)
    A = np.ascontiguousarray(Wd[idx])  # [3072, 768]
    return np.ascontiguousarray(
        A.reshape(FC, P, NKC, P).transpose(0, 3, 2, 1)
    ).astype(_BF)


def _prep_hsT(hs_b):
    # [512, 768] -> [768,512] -> (ki, ko, s)
    return np.ascontiguousarray(
        hs_b.T.reshape(NKC, P, S).transpose(1, 0, 2)
    ).astype(_BF)


def _prep_wv(Wv, h0):
    Wvs = Wv[h0 * HD : (h0 + HB) * HD]  # [384, 768]
    return np.ascontiguousarray(
        Wvs.T.reshape(NKC, P, HB * HD).transpose(1, 0, 2)
    ).astype(_BF)


last_results = None


def kernel(
    hidden_states,
    attention_mask,
    Wq,
    bq,
    Wk,
    bk,
    Wv,
    bv,
    Wqd,
    bqd,
    Wkd,
    bkd,
):
    global last_results
    from concourse.bass_utils import run_bass_kernel_spmd

    hs = np.asarray(hidden_states, dtype=np.float32)
    mask = np.asarray(attention_mask)
    Wv = np.asarray(Wv, dtype=np.float32)
    bv = np.asarray(bv, dtype=np.float32)
    Wqd = np.asarray(Wqd, dtype=np.float32)
    bqd = np.asarray(bqd, dtype=np.float32)
    Wkd = np.asarray(Wkd, dtype=np.float32)
    bkd = np.asarray(bkd, dtype=np.float32)

    use_mask = not bool(np.all(mask != 0))
    use_qk_bias = bool(np.any(bqd) or np.any(bkd))
    nc = _get_program(use_mask, use_qk_bias)

    idx = [_perm_idx(0), _perm_idx(HB)]
    wq_p = [_prep_w(Wqd, idx[hh]) for hh in range(2)]
    wk_p = [_prep_w(Wkd, idx[hh]) for hh in range(2)]
    wv_p = [_prep_wv(Wv, hh * HB) for hh in range(2)]
    qb_p = [
        np.ascontiguousarray(bqd[idx[hh]].reshape(FC, P).T).astype(np.float32)
        for hh in range(2)
    ]
    kb_p = [
        np.ascontiguousarray(bkd[idx[hh]].reshape(FC, P).T).astype(np.float32)
        for hh in range(2)
    ]

    in_maps = []
    for c in range(N_CORES):
        b, hh = c // 2, c % 2
        m = {
            "hsT": _prep_hsT(hs[b]),
            "wq": wq_p[hh],
            "wk": wk_p[hh],
            "wv": wv_p[hh],
        }
        if use_qk_bias:
            m["qb"] = qb_p[hh]
            m["kb"] = kb_p[hh]
        if use_mask:
            mb = np.where(mask[b] == 0, np.float32(-1e9), np.float32(0.0))
            m["mb"] = np.ascontiguousarray(
                mb.reshape(KSC, P).T
            ).astype(np.float32)
        in_maps.append(m)

    res = run_bass_kernel_spmd(nc, in_maps, list(range(N_CORES)))
    last_results = res

    out = np.empty((DEPTH, B, HEADS, S, HD), dtype=np.float32)
    for c in range(N_CORES):
        b, hh = c // 2, c % 2
        arr = res.results[c]["out"]  # [6, 2, 128, 4, 4, 64] bf16
        a = (
            np.ascontiguousarray(arr.transpose(1, 4, 0, 3, 2, 5))
            .astype(np.float32)
            .reshape(DEPTH, HB, S, HD)
        )
        out[:, b, hh * HB : (hh + 1) * HB] = a
    if np.any(bv):
        out += bv.reshape(HEADS, HD)[None, None, :, None, :]
    return out

